# revision 9
# baseline (speedup 1.0000x reference)
"""Graves attention (GMM attention) Trainium2 kernel.

Shapes (hardcoded from the problem spec):
  B=1024, T=500, D=256, K=20, F=512 (mem_elem), H=51 (hidden), 3K=60.

Sharding: pure data-parallel over batch across 8 NeuronCores
(128 rows/core).  Per core:
  x = C.reshape(128, 512)
  h = relu(x @ W1 + b1)                       # PE matmuls (x transposed on PE)
  gbk = h @ W2 + b2                           # PE
  g = softmax(gbk[:, 0:20]) + EPS             # ACT/DVE
  sig = exp(gbk[:, 20:40]) + EPS
  mu = mu_tm1 + 0.05 * exp(gbk[:, 40:60])
  alpha[b,t] = sum_k COEF*g[b,k]*exp(-0.5*sig[b,k]*(t-mu[b,k])^2)
     -> per k: ACT Square(iota, bias=-mu_k); ACT Exp(scale=-0.5sig_k,
        bias=ln(COEF*g_k)); DVE accumulate.  (folding COEF*g into the
        Exp bias makes each k 2 ACT ops + 1 DVE op on (128,500))
  c[b,:] = alpha[b,:] @ ctx[b]                # per-row PE matmuls, alpha
                                              # chunks stationary, fp32r
The 64MB/core context stream is the roofline (~358 GB/s/NC -> ~180us).
"""

import numpy as np

B = 1024
NCORES = 8
BS = B // NCORES          # 128 batch rows per core
T = 500
D = 256
K = 20
F = 512
H = 51
G3 = 3 * K                # 60
TC = 125                  # t-chunk for c_t matmuls (500 = 4*125)
NCH = 4
COEF = 0.3989422917366028
EPS = 1e-05
ALIGN = 0.05

_CACHE = {}


def _build_program():
    import concourse.bass as bass
    import concourse.bacc as bacc
    import concourse.tile as tile
    import concourse.mybir as mybir
    from contextlib import ExitStack

    FP = mybir.dt.float32
    FPR = mybir.dt.float32r
    AF = mybir.ActivationFunctionType
    OP = mybir.AluOpType
    AX = mybir.AxisListType

    nc = bacc.Bacc(
        "TRN2",
        target_bir_lowering=False,
        debug=False,
        enable_asserts=False,
        num_devices=NCORES,
    )

    C_d = nc.dram_tensor("C", [BS, F], FP, kind="ExternalInput")
    ctx_d = nc.dram_tensor("ctx", [BS, T, D], FPR, kind="ExternalInput")
    mu_d = nc.dram_tensor("mu", [BS, K], FP, kind="ExternalInput")
    W1_d = nc.dram_tensor("W1", [F, H], FP, kind="ExternalInput")
    b1_d = nc.dram_tensor("b1", [H, 1], FP, kind="ExternalInput")
    W2_d = nc.dram_tensor("W2", [H, G3], FP, kind="ExternalInput")
    b2_d = nc.dram_tensor("b2", [G3, 1], FP, kind="ExternalInput")
    iota_d = nc.dram_tensor("iota", [BS, T], FP, kind="ExternalInput")
    eye_d = nc.dram_tensor("eye", [128, 128], FP, kind="ExternalInput")

    ct_d = nc.dram_tensor("ct", [BS, D], FP, kind="ExternalOutput")
    muo_d = nc.dram_tensor("mu_out", [BS, K], FP, kind="ExternalOutput")
    al_d = nc.dram_tensor("alpha", [BS, T], FP, kind="ExternalOutput")

    with tile.TileContext(nc) as tc:
        with ExitStack() as es:
            const = es.enter_context(tc.tile_pool(name="const", bufs=1))
            work = es.enter_context(tc.tile_pool(name="work", bufs=2))
            ps = es.enter_context(
                tc.tile_pool(name="ps", bufs=2, space=bass.MemorySpace.PSUM)
            )
            psr = es.enter_context(
                tc.tile_pool(name="psr", bufs=4, space=bass.MemorySpace.PSUM)
            )
            rows = es.enter_context(tc.tile_pool(name="rows", bufs=16))

            # ---- constant loads ----
            iota_t = const.tile([BS, T], FP)
            nc.sync.dma_start(iota_t[:], iota_d[:])
            eye_t = const.tile([128, 128], FP)
            nc.sync.dma_start(eye_t[:], eye_d[:])
            w1_t = const.tile([128, NCH, H], FP)
            for c in range(NCH):
                nc.sync.dma_start(w1_t[:, c, :], W1_d[c * 128:(c + 1) * 128, :])
            w2_t = const.tile([H, G3], FP)
            nc.sync.dma_start(w2_t[:], W2_d[:])
            b1_t = const.tile([H, 1], FP)
            nc.sync.dma_start(b1_t[:], b1_d[:])
            b2_t = const.tile([G3, 1], FP)
            nc.sync.dma_start(b2_t[:], b2_d[:])
            mu_in = const.tile([BS, K], FP)
            nc.sync.dma_start(mu_in[:], mu_d[:])
            x_t = const.tile([BS, F], FP)
            nc.sync.dma_start(x_t[:], C_d[:])

            # ---- transpose x (PE) ----
            xT = const.tile([128, NCH, 128], FP)  # [f_local, chunk, b]
            for c in range(NCH):
                tp = ps.tile([128, 128], FP, tag="tp")
                nc.tensor.transpose(tp[:], x_t[:, c * 128:(c + 1) * 128], eye_t[:])
                nc.vector.tensor_copy(xT[:, c, :], tp[:])

            # ---- MLP ----
            h_ps = ps.tile([H, BS], FP, tag="mlp")
            for c in range(NCH):
                nc.tensor.matmul(
                    h_ps[:], w1_t[:, c, :], xT[:, c, :],
                    start=(c == 0), stop=(c == NCH - 1),
                )
            hT = const.tile([H, BS], FP)
            # max(h + b1, 0)
            nc.vector.tensor_scalar(hT[:], h_ps[:], b1_t[:], 0.0, OP.add, OP.max)

            g_ps = ps.tile([G3, BS], FP, tag="mlp")
            nc.tensor.matmul(g_ps[:], w2_t[:], hT[:], start=True, stop=True)
            gT = const.tile([G3, BS], FP)
            nc.vector.tensor_scalar(gT[:], g_ps[:], b2_t[:], None, OP.add)

            gb_ps = ps.tile([128, G3], FP, tag="tp")
            nc.tensor.transpose(gb_ps[:], gT[:], eye_t[:G3, :G3])
            gbk = const.tile([BS, G3], FP)
            nc.vector.tensor_copy(gbk[:], gb_ps[:])

            # ---- softmax / gates ----
            mx = const.tile([BS, 1], FP)
            nc.vector.reduce_max(mx[:], gbk[:, 0:K], axis=AX.X)
            negmx = const.tile([BS, 1], FP)
            nc.vector.tensor_scalar_mul(negmx[:], mx[:], -1.0)
            e0 = const.tile([BS, K], FP)
            ssum = const.tile([BS, 1], FP)
            nc.scalar.activation(e0[:], gbk[:, 0:K], AF.Exp, bias=negmx[:],
                                 scale=1.0, accum_out=ssum[:])
            rs = const.tile([BS, 1], FP)
            nc.vector.reciprocal(rs[:], ssum[:])
            g_t = const.tile([BS, K], FP)
            nc.vector.tensor_scalar(g_t[:], e0[:], rs[:], EPS, OP.mult, OP.add)
            lg = const.tile([BS, K], FP)
            # ln(COEF * g)
            nc.scalar.activation(lg[:], g_t[:], AF.Ln, scale=COEF)

            es_t = const.tile([BS, K], FP)
            nc.scalar.activation(es_t[:], gbk[:, K:2 * K], AF.Exp)
            nhs = const.tile([BS, K], FP)
            # -0.5 * (exp + EPS)
            nc.vector.tensor_scalar(nhs[:], es_t[:], EPS, -0.5, OP.add, OP.mult)

            em = const.tile([BS, K], FP)
            nc.scalar.activation(em[:], gbk[:, 2 * K:3 * K], AF.Exp)
            dmu = const.tile([BS, K], FP)
            nc.vector.tensor_scalar_mul(dmu[:], em[:], ALIGN)
            mu_new = const.tile([BS, K], FP)
            nc.vector.tensor_tensor(mu_new[:], dmu[:], mu_in[:], OP.add)
            negmu = const.tile([BS, K], FP)
            nc.vector.tensor_scalar_mul(negmu[:], mu_new[:], -1.0)
            nc.sync.dma_start(muo_d[:], mu_new[:])

            # ---- alpha accumulation over K mixture components ----
            acc = const.tile([BS, T], FP)
            for k in range(K):
                sq = work.tile([BS, T], FP, tag="sq")
                nc.scalar.activation(sq[:], iota_t[:], AF.Square,
                                     bias=negmu[:, k:k + 1])
                if k == 0:
                    nc.scalar.activation(acc[:], sq[:], AF.Exp,
                                         bias=lg[:, k:k + 1],
                                         scale=nhs[:, k:k + 1])
                else:
                    term = work.tile([BS, T], FP, tag="term")
                    nc.scalar.activation(term[:], sq[:], AF.Exp,
                                         bias=lg[:, k:k + 1],
                                         scale=nhs[:, k:k + 1])
                    nc.vector.tensor_tensor(acc[:], acc[:], term[:], OP.add)
            nc.sync.dma_start(al_d[:], acc[:])

            # ---- alpha transposed for per-row matmuls ----
            # padded to 130 columns so the (125, 2) stationary slice at
            # b=127 stays in bounds (fp32r needs an even free count)
            aT = const.tile([TC, NCH, 130], FPR)  # [t_local, chunk, b]
            for c in range(NCH):
                nc.gpsimd.memset(aT[:, c, 128:130].bitcast(FP), 0.0)
                tpa = ps.tile([128, 128], FP, tag="tp")
                nc.tensor.transpose(tpa[:TC, :], acc[:, c * TC:(c + 1) * TC],
                                    eye_t[:])
                nc.vector.tensor_copy(aT[:, c, 0:128], tpa[:TC, :])

            # ---- c_t: per-row alpha @ ctx ----
            # alpha chunk (padded to 2 stationary columns for fp32r's
            # even-free-count rule) is stationary; the ctx chunk (125,256)
            # is moving.  psum row 0 is the real result, row 1 is junk.
            for b in range(BS):
                rt = rows.tile([TC, NCH, D], FPR, tag="rt")
                nc.sync.dma_start(
                    rt[:], ctx_d[b].rearrange("(c t) d -> t c d", c=NCH))
                pr = psr.tile([2, D], FP, tag="pr")
                for c in range(NCH):
                    nc.tensor.matmul(
                        pr[:],
                        aT[:, c, b:b + 2],
                        rt[:, c, :],
                        start=(c == 0), stop=(c == NCH - 1),
                    )
                rowbuf = rows.tile([1, D], FP, tag="rowbuf", bufs=8)
                if b % 2 == 0:
                    nc.scalar.copy(rowbuf[:], pr[0:1, :])
                else:
                    nc.vector.tensor_copy(rowbuf[:], pr[0:1, :])
                nc.sync.dma_start(ct_d[b:b + 1, :], rowbuf[:])

    nc.compile()
    return nc


def _get_nc():
    if "nc" not in _CACHE:
        _CACHE["nc"] = _build_program()
    return _CACHE["nc"]


def _make_in_maps(C, context, mu_tm1, W1, b1, W2, b2):
    C = np.ascontiguousarray(np.asarray(C, dtype=np.float32)).reshape(B, F)
    context = np.asarray(context, dtype=np.float32)
    mu_tm1 = np.asarray(mu_tm1, dtype=np.float32)
    W1 = np.ascontiguousarray(np.asarray(W1, dtype=np.float32))
    b1 = np.ascontiguousarray(np.asarray(b1, dtype=np.float32)).reshape(H, 1)
    W2 = np.ascontiguousarray(np.asarray(W2, dtype=np.float32))
    b2 = np.ascontiguousarray(np.asarray(b2, dtype=np.float32)).reshape(G3, 1)
    iota = np.ascontiguousarray(
        np.broadcast_to(np.arange(T, dtype=np.float32), (BS, T)))
    eye = np.eye(128, dtype=np.float32)

    in_maps = []
    for c in range(NCORES):
        sl = slice(c * BS, (c + 1) * BS)
        in_maps.append({
            "C": np.ascontiguousarray(C[sl]),
            "ctx": np.ascontiguousarray(context[sl]),
            "mu": np.ascontiguousarray(mu_tm1[sl]),
            "W1": W1,
            "b1": b1,
            "W2": W2,
            "b2": b2,
            "iota": iota,
            "eye": eye,
        })
    return in_maps


def _gather(results):
    ct = np.concatenate([results[c]["ct"] for c in range(NCORES)], axis=0)
    mu = np.concatenate([results[c]["mu_out"] for c in range(NCORES)], axis=0)
    al = np.concatenate([results[c]["alpha"] for c in range(NCORES)], axis=0)
    return ct, mu, al.reshape(B, 1, T)


def _run(in_maps, trace=False, **kwargs):
    from concourse.bass_utils import run_bass_kernel_spmd

    nc = _get_nc()
    return run_bass_kernel_spmd(nc, in_maps, core_ids=list(range(NCORES)),
                                trace=trace, **kwargs)


def kernel(C, context, mu_tm1, W1, b1, W2, b2):
    in_maps = _make_in_maps(C, context, mu_tm1, W1, b1, W2, b2)
    res = _run(in_maps, trace=False)
    return _gather(res.results)


# revision 10
# speedup vs baseline: 1.1237x; 1.1237x over previous
"""Graves attention (GMM attention) Trainium2 kernel.

Shapes (hardcoded from the problem spec):
  B=1024, T=500, D=256, K=20, F=512 (mem_elem), H=51 (hidden), 3K=60.

Sharding: pure data-parallel over batch across 8 NeuronCores
(128 rows/core).  Per core:
  x = C.reshape(128, 512)
  h = relu(x @ W1 + b1)                       # PE matmuls (x transposed on PE)
  gbk = h @ W2 + b2                           # PE
  g = softmax(gbk[:, 0:20]) + EPS             # ACT/DVE
  sig = exp(gbk[:, 20:40]) + EPS
  mu = mu_tm1 + 0.05 * exp(gbk[:, 40:60])
  alpha[b,t] = sum_k COEF*g[b,k]*exp(-0.5*sig[b,k]*(t-mu[b,k])^2)
     -> per k: ACT Square(iota, bias=-mu_k); ACT Exp(scale=-0.5sig_k,
        bias=ln(COEF*g_k)); DVE accumulate.  (folding COEF*g into the
        Exp bias makes each k 2 ACT ops + 1 DVE op on (128,500))
  c[b,:] = alpha[b,:] @ ctx[b]                # per-row PE matmuls, alpha
                                              # chunks stationary, fp32r
The 64MB/core context stream is the roofline (~358 GB/s/NC -> ~180us).
"""

import numpy as np

B = 1024
NCORES = 8
BS = B // NCORES          # 128 batch rows per core
T = 500
D = 256
K = 20
F = 512
H = 51
G3 = 3 * K                # 60
TC = 125                  # t-chunk for c_t matmuls (500 = 4*125)
NCH = 4
COEF = 0.3989422917366028
EPS = 1e-05
ALIGN = 0.05

_CACHE = {}


def _build_program():
    import concourse.bass as bass
    import concourse.bacc as bacc
    import concourse.tile as tile
    import concourse.mybir as mybir
    from contextlib import ExitStack

    FP = mybir.dt.float32
    FPR = mybir.dt.float32r
    AF = mybir.ActivationFunctionType
    OP = mybir.AluOpType
    AX = mybir.AxisListType

    nc = bacc.Bacc(
        "TRN2",
        target_bir_lowering=False,
        debug=False,
        enable_asserts=False,
        num_devices=NCORES,
    )

    C_d = nc.dram_tensor("C", [BS, F], FP, kind="ExternalInput")
    ctx_d = nc.dram_tensor("ctx", [BS, T, D], FPR, kind="ExternalInput")
    mu_d = nc.dram_tensor("mu", [BS, K], FP, kind="ExternalInput")
    W1_d = nc.dram_tensor("W1", [F, H], FP, kind="ExternalInput")
    b1_d = nc.dram_tensor("b1", [H, 1], FP, kind="ExternalInput")
    W2_d = nc.dram_tensor("W2", [H, G3], FP, kind="ExternalInput")
    b2_d = nc.dram_tensor("b2", [G3, 1], FP, kind="ExternalInput")
    iota_d = nc.dram_tensor("iota", [BS, T], FP, kind="ExternalInput")
    eye_d = nc.dram_tensor("eye", [128, 128], FP, kind="ExternalInput")

    ct_d = nc.dram_tensor("ct", [BS, D], FP, kind="ExternalOutput")
    muo_d = nc.dram_tensor("mu_out", [BS, K], FP, kind="ExternalOutput")
    al_d = nc.dram_tensor("alpha", [BS, T], FP, kind="ExternalOutput")

    with tile.TileContext(nc) as tc:
        with ExitStack() as es:
            const = es.enter_context(tc.tile_pool(name="const", bufs=1))
            work = es.enter_context(tc.tile_pool(name="work", bufs=2))
            ps = es.enter_context(
                tc.tile_pool(name="ps", bufs=2, space=bass.MemorySpace.PSUM)
            )
            psr = es.enter_context(
                tc.tile_pool(name="psr", bufs=4, space=bass.MemorySpace.PSUM)
            )
            rows = es.enter_context(tc.tile_pool(name="rows", bufs=16))

            # ---- constant loads ----
            iota_t = const.tile([BS, T], FP)
            nc.sync.dma_start(iota_t[:], iota_d[:])
            eye_t = const.tile([128, 128], FP)
            nc.sync.dma_start(eye_t[:], eye_d[:])
            w1_t = const.tile([128, NCH, H], FP)
            for c in range(NCH):
                nc.sync.dma_start(w1_t[:, c, :], W1_d[c * 128:(c + 1) * 128, :])
            w2_t = const.tile([H, G3], FP)
            nc.sync.dma_start(w2_t[:], W2_d[:])
            b1_t = const.tile([H, 1], FP)
            nc.sync.dma_start(b1_t[:], b1_d[:])
            b2_t = const.tile([G3, 1], FP)
            nc.sync.dma_start(b2_t[:], b2_d[:])
            mu_in = const.tile([BS, K], FP)
            nc.sync.dma_start(mu_in[:], mu_d[:])
            x_t = const.tile([BS, F], FP)
            nc.sync.dma_start(x_t[:], C_d[:])

            # ---- transpose x (PE) ----
            xT = const.tile([128, NCH, 128], FP)  # [f_local, chunk, b]
            for c in range(NCH):
                tp = ps.tile([128, 128], FP, tag="tp")
                nc.tensor.transpose(tp[:], x_t[:, c * 128:(c + 1) * 128], eye_t[:])
                nc.vector.tensor_copy(xT[:, c, :], tp[:])

            # ---- MLP ----
            h_ps = ps.tile([H, BS], FP, tag="mlp")
            for c in range(NCH):
                nc.tensor.matmul(
                    h_ps[:], w1_t[:, c, :], xT[:, c, :],
                    start=(c == 0), stop=(c == NCH - 1),
                )
            hT = const.tile([H, BS], FP)
            # max(h + b1, 0)
            nc.vector.tensor_scalar(hT[:], h_ps[:], b1_t[:], 0.0, OP.add, OP.max)

            g_ps = ps.tile([G3, BS], FP, tag="mlp")
            nc.tensor.matmul(g_ps[:], w2_t[:], hT[:], start=True, stop=True)
            gT = const.tile([G3, BS], FP)
            nc.vector.tensor_scalar(gT[:], g_ps[:], b2_t[:], None, OP.add)

            gb_ps = ps.tile([128, G3], FP, tag="tp")
            nc.tensor.transpose(gb_ps[:], gT[:], eye_t[:G3, :G3])
            gbk = const.tile([BS, G3], FP)
            nc.vector.tensor_copy(gbk[:], gb_ps[:])

            # ---- softmax / gates ----
            mx = const.tile([BS, 1], FP)
            nc.vector.reduce_max(mx[:], gbk[:, 0:K], axis=AX.X)
            negmx = const.tile([BS, 1], FP)
            nc.vector.tensor_scalar_mul(negmx[:], mx[:], -1.0)
            e0 = const.tile([BS, K], FP)
            ssum = const.tile([BS, 1], FP)
            nc.scalar.activation(e0[:], gbk[:, 0:K], AF.Exp, bias=negmx[:],
                                 scale=1.0, accum_out=ssum[:])
            rs = const.tile([BS, 1], FP)
            nc.vector.reciprocal(rs[:], ssum[:])
            g_t = const.tile([BS, K], FP)
            nc.vector.tensor_scalar(g_t[:], e0[:], rs[:], EPS, OP.mult, OP.add)
            lg = const.tile([BS, K], FP)
            # ln(COEF * g)
            nc.scalar.activation(lg[:], g_t[:], AF.Ln, scale=COEF)

            es_t = const.tile([BS, K], FP)
            nc.scalar.activation(es_t[:], gbk[:, K:2 * K], AF.Exp)
            nhs = const.tile([BS, K], FP)
            # -0.5 * (exp + EPS)
            nc.vector.tensor_scalar(nhs[:], es_t[:], EPS, -0.5, OP.add, OP.mult)

            em = const.tile([BS, K], FP)
            nc.scalar.activation(em[:], gbk[:, 2 * K:3 * K], AF.Exp)
            dmu = const.tile([BS, K], FP)
            nc.vector.tensor_scalar_mul(dmu[:], em[:], ALIGN)
            mu_new = const.tile([BS, K], FP)
            nc.vector.tensor_tensor(mu_new[:], dmu[:], mu_in[:], OP.add)
            negmu = const.tile([BS, K], FP)
            nc.vector.tensor_scalar_mul(negmu[:], mu_new[:], -1.0)
            nc.scalar.dma_start(muo_d[:], mu_new[:])

            # ---- alpha accumulation over K mixture components ----
            acc = const.tile([BS, T], FP)
            for k in range(K):
                sq = work.tile([BS, T], FP, tag="sq")
                nc.scalar.activation(sq[:], iota_t[:], AF.Square,
                                     bias=negmu[:, k:k + 1])
                if k == 0:
                    nc.scalar.activation(acc[:], sq[:], AF.Exp,
                                         bias=lg[:, k:k + 1],
                                         scale=nhs[:, k:k + 1])
                else:
                    term = work.tile([BS, T], FP, tag="term")
                    nc.scalar.activation(term[:], sq[:], AF.Exp,
                                         bias=lg[:, k:k + 1],
                                         scale=nhs[:, k:k + 1])
                    nc.vector.tensor_tensor(acc[:], acc[:], term[:], OP.add)
            acc_nat = const.tile([BS, T], FP)
            nc.vector.tensor_copy(
                acc_nat.rearrange("b (j s) -> b s j", s=4),
                acc.rearrange("b (s j) -> b s j", j=TC))
            nc.scalar.dma_start(al_d[:], acc_nat[:])

            # ---- alpha transposed for per-row matmuls ----
            # padded to 130 columns so the (125, 2) stationary slice at
            # b=127 stays in bounds (fp32r needs an even free count)
            aT = const.tile([TC, NCH, 130], FPR)  # [t_local, chunk, b]
            for c in range(NCH):
                nc.gpsimd.memset(aT[:, c, 128:130].bitcast(FP), 0.0)
                tpa = ps.tile([128, 128], FP, tag="tp")
                nc.tensor.transpose(tpa[:TC, :], acc[:, c * TC:(c + 1) * TC],
                                    eye_t[:])
                nc.vector.tensor_copy(aT[:, c, 0:128], tpa[:TC, :])

            # ---- c_t: per-row alpha @ ctx ----
            # alpha chunk (padded to 2 stationary columns for fp32r's
            # even-free-count rule) is stationary; the ctx chunk (125,256)
            # is moving.  psum row 0 is the real result, row 1 is junk.
            for b in range(BS):
                rt = rows.tile([TC, NCH, D], FPR, tag="rt")
                nc.sync.dma_start(
                    rt[:], ctx_d[b].rearrange("(t s) d -> t s d", s=NCH))
                pr = psr.tile([2, D], FP, tag="pr")
                for c in range(NCH):
                    nc.tensor.matmul(
                        pr[:],
                        aT[:, c, b:b + 2],
                        rt[:, c, :],
                        start=(c == 0), stop=(c == NCH - 1),
                    )
                rowbuf = rows.tile([1, D], FP, tag="rowbuf", bufs=8)
                if b % 2 == 0:
                    nc.scalar.copy(rowbuf[:], pr[0:1, :])
                else:
                    nc.vector.tensor_copy(rowbuf[:], pr[0:1, :])
                nc.scalar.dma_start(ct_d[b:b + 1, :], rowbuf[:])

    nc.compile()
    return nc


def _get_nc():
    if "nc" not in _CACHE:
        _CACHE["nc"] = _build_program()
    return _CACHE["nc"]


def _make_in_maps(C, context, mu_tm1, W1, b1, W2, b2):
    C = np.ascontiguousarray(np.asarray(C, dtype=np.float32)).reshape(B, F)
    context = np.asarray(context, dtype=np.float32)
    mu_tm1 = np.asarray(mu_tm1, dtype=np.float32)
    W1 = np.ascontiguousarray(np.asarray(W1, dtype=np.float32))
    b1 = np.ascontiguousarray(np.asarray(b1, dtype=np.float32)).reshape(H, 1)
    W2 = np.ascontiguousarray(np.asarray(W2, dtype=np.float32))
    b2 = np.ascontiguousarray(np.asarray(b2, dtype=np.float32)).reshape(G3, 1)
    iota_perm = np.concatenate([np.arange(s, T, 4) for s in range(4)]).astype(np.float32)
    iota = np.ascontiguousarray(np.broadcast_to(iota_perm, (BS, T)))
    eye = np.eye(128, dtype=np.float32)

    in_maps = []
    for c in range(NCORES):
        sl = slice(c * BS, (c + 1) * BS)
        in_maps.append({
            "C": np.ascontiguousarray(C[sl]),
            "ctx": np.ascontiguousarray(context[sl]),
            "mu": np.ascontiguousarray(mu_tm1[sl]),
            "W1": W1,
            "b1": b1,
            "W2": W2,
            "b2": b2,
            "iota": iota,
            "eye": eye,
        })
    return in_maps


def _gather(results):
    ct = np.concatenate([results[c]["ct"] for c in range(NCORES)], axis=0)
    mu = np.concatenate([results[c]["mu_out"] for c in range(NCORES)], axis=0)
    al = np.concatenate([results[c]["alpha"] for c in range(NCORES)], axis=0)
    return ct, mu, al.reshape(B, 1, T)


def _run(in_maps, trace=False, **kwargs):
    from concourse.bass_utils import run_bass_kernel_spmd

    nc = _get_nc()
    return run_bass_kernel_spmd(nc, in_maps, core_ids=list(range(NCORES)),
                                trace=trace, **kwargs)


def kernel(C, context, mu_tm1, W1, b1, W2, b2):
    in_maps = _make_in_maps(C, context, mu_tm1, W1, b1, W2, b2)
    res = _run(in_maps, trace=False)
    return _gather(res.results)
